# revision 11
# baseline (speedup 1.0000x reference)
"""Trainium2 Bass kernel for nn_CLTBernoulliDecoder (CLT Bernoulli decoder loss).

Reference computation:
    logits = (z @ W + b).reshape(Bz, F, 2)        # (j, s) column-interleaved
    root fix: logits[:, root, 0] := logits[:, root, 1]
    xt = x[:, tree] ;  x_cond = stack([1-xt, xt])
    ls, lsn = log_sigmoid(+-logits)
    out[b,i] = sum_{j,s} x_cond*x * ls + x_cond*(1-x) * lsn

Algebraic restructuring (exact):
    log_sigmoid(t) = t - softplus(t), log_sigmoid(-t) = -softplus(t)
    =>  out[b,i] = sum_r Ahat[b,r]*l_r[i]  -  sum_r xc[b,r]*softplus(l_r[i])
    over flat rows r = 2j+s (natural W column order), with
    xc[b,2j+s] = x_cond[b,j,s], Ahat = xc*x, root fix folded as xt'=1.
    The linear term folds through W: G = Ahat@W.T.

Activation split across two engines:
    softplus(l) = ln2 + l/2 + g(l),  g(l) = ln(cosh(l/2)) even in l.
  - ACT rows (tiles 0..6): softplus via Exp then Ln(1+e), emitted in fp8.
  - DVE rows (tiles 7..12): g(l) ~= (c2*u + c1)*u with u = l*l
    (cast from PSUM + square / affine / multiply in 2x/4x bf16 modes);
    the (ln2 + l/2) part is folded host-side into Ahat (x -> x-1/2) and
    a -ln2*n constant into h.  Fit |l| <= 2.59: max err 6e-3, typ 1.5e-4.

Tensor engine in fp8 where it is cheap:
  - logits matmuls use fp8e4m3 z/W in DoubleRow layout [33, 2, *]
    (rows 2p+k; row 64 = bias ones row, row 65 = zero pad) -> 2 rows/cycle.
  - ACT-tile main matmuls contract fp8 sp against fp8 coefficients with
    DoubleRow over tile pairs; DVE-tile mains stay bf16 (g is produced by
    the DVE, which has no fast fp8 write path).
  - The linear term gp.T @ zp stays bf16 for accuracy (h is large).

Sharding: data-parallel over Bz (4096 -> 8 x 512); coefficient matrices
replicated; per-core outputs [256, 512] concatenated on axis 1.
"""

import numpy as np
import ml_dtypes

BF16 = ml_dtypes.bfloat16
FP8 = ml_dtypes.float8_e4m3

BX = 256          # data points
BZ = 4096         # latent samples
ZD = 64           # latent dim
F = 784           # features
R = 2 * F         # flat (j, s) rows = 1568
NT = 13           # row tiles of 128 (1664 padded)
RP = NT * 128
KD = (ZD + 2) // 2  # 33 DoubleRow contraction partitions (rows 0..65)
N_CORES = 8
BZS = BZ // N_CORES  # 512 per core

N_ACT_TILES = 7            # tiles 0..6 -> ACT engine (exp+ln)
DVE_T0 = N_ACT_TILES       # tiles 7..12 -> DVE engine (poly even part)
# g(l) = ln(cosh(l/2)) ~= (C2*u + C1)*u, u = l^2, fit for |l| <= 2.59
C1 = 0.12345821
C2 = -0.00355909

_CACHE = {}


def _build_bass():
    import concourse.bass as bass
    import concourse.mybir as mybir
    import concourse.tile as tile
    from concourse import bacc
    from concourse.hw_specs import get_activation_tables

    fp32 = mybir.dt.float32
    bf16 = mybir.dt.bfloat16
    fp8 = mybir.dt.float8e4
    EXP = mybir.ActivationFunctionType.Exp
    LN = mybir.ActivationFunctionType.Ln
    MULT = mybir.AluOpType.mult
    ADD = mybir.AluOpType.add
    DR = mybir.MatmulPerfMode.DoubleRow

    class _Bacc(bacc.Bacc):
        """Pin Exp and Ln to the one table set holding both, so the table
        is loaded once instead of ping-ponging between per-function sets
        (~1.3us per reload). Table ids are global act_info indices, so the
        full table list must be kept in order."""

        def insert_act_table_loads(self):
            EXPF = mybir.ActivationFunctionType.Exp
            LNF = mybir.ActivationFunctionType.Ln
            has_activation = any(
                isinstance(i, mybir.InstActivation)
                for b in self.main_func.blocks
                for i in b.instructions
            )
            if not has_activation:
                return
            tables = []
            for name, funcs in get_activation_tables(self.m.arch).items():
                if name != "natural_log_exp_and_others":
                    funcs = {f for f in funcs if f not in (EXPF, LNF)}
                tables.append((name, funcs))
            import bass_rust as _bass_rust
            _bass_rust.insert_act_table_loads(self, tables)

    nc = _Bacc(None, target_bir_lowering=False)

    d_zq = nc.dram_tensor("zq", [KD, 2, BZS], fp8, kind="ExternalInput")
    d_zp = nc.dram_tensor("zp", [ZD + 1, BZS], bf16, kind="ExternalInput")
    d_wq0 = nc.dram_tensor("wq0", [KD, 2, 256], fp8, kind="ExternalInput")
    d_wqr = nc.dram_tensor("wqr", [KD, 2, RP - 256], fp8, kind="ExternalInput")
    d_gp = nc.dram_tensor("gp", [ZD + 1, BX], bf16, kind="ExternalInput")
    d_uva = nc.dram_tensor("uva", [128, N_ACT_TILES, BX], fp8,
                           kind="ExternalInput")
    d_uvd = nc.dram_tensor("uvd", [128, NT - N_ACT_TILES, BX], bf16,
                           kind="ExternalInput")
    d_out = nc.dram_tensor("out", [BX, BZS], fp32, kind="ExternalOutput")

    # tile groups: (tiles, psum_tag, engine)
    GROUPS = [
        ((0, 1), "pA", "act"),
        ((7, 8), "pB", "dve"),
        ((2, 3), "pC", "act"),
        ((9, 10), "pA", "dve"),
        ((11, 12), "pB", "dve"),
        ((4, 5), "pC", "act"),
        ((6,), "pA", "act"),
    ]

    with tile.TileContext(nc) as tc:
        with (
            tc.tile_pool(name="singles", bufs=1) as singles,
            tc.tile_pool(name="psum_l", bufs=1, space="PSUM") as psum_l,
            tc.tile_pool(name="psum_o", bufs=1, space="PSUM") as psum_o,
        ):
            # ---- SBUF staging ----
            wu = singles.tile([128, 256], bf16)
            zq = singles.tile([KD, 2, BZS], fp8)
            zp = singles.tile([ZD + 1, BZS], bf16)
            wq0 = singles.tile([KD, 2, 256], fp8)
            wqr = singles.tile([KD, 2, RP - 256], fp8)
            gp = singles.tile([ZD + 1, BX], bf16)
            uva = singles.tile([128, N_ACT_TILES, BX], fp8)
            uvd = singles.tile([128, NT - N_ACT_TILES, BX], bf16)
            e_all = singles.tile([128, N_ACT_TILES, BZS], fp32)
            lb_all = singles.tile([128, NT - N_ACT_TILES, BZS], bf16)
            u_all = singles.tile([128, NT - N_ACT_TILES, BZS], bf16)
            r_all = singles.tile([128, NT - N_ACT_TILES, BZS], bf16)
            spa = singles.tile([128, N_ACT_TILES, BZS], fp8)
            spd = singles.tile([128, NT - N_ACT_TILES, BZS], bf16)
            e_flat = e_all.rearrange("p t i -> p (t i)")
            lb_flat = lb_all.rearrange("p t i -> p (t i)")
            u_flat = u_all.rearrange("p t i -> p (t i)")
            r_flat = r_all.rearrange("p t i -> p (t i)")
            spa_flat = spa.rearrange("p t i -> p (t i)")
            spd_flat = spd.rearrange("p t i -> p (t i)")

            # ---- input DMAs ----
            nc.vector.memset(wu, 0.0)  # DVE is idle: warmup unblocks first
            nc.sync.dma_start(out=zq, in_=d_zq[:])
            nc.scalar.dma_start(out=wq0, in_=d_wq0[:])
            nc.sync.dma_start(out=zp, in_=d_zp[:])
            nc.scalar.dma_start(out=wqr, in_=d_wqr[:])
            nc.gpsimd.dma_start(out=gp, in_=d_gp[:])
            nc.gpsimd.dma_start(out=uva, in_=d_uva[:])
            nc.gpsimd.dma_start(out=uvd, in_=d_uvd[:])

            # ---- PE warm-up while DMAs land (trips the HAM clock gate) ----
            wu_ps = psum_l.tile([128, 2 * BZS], fp32, tag="pC", name="wu_ps")
            for _ in range(6):
                nc.tensor.matmul(wu_ps[:, 0:256], wu[:, 0:128], wu,
                                 start=True, stop=True)

            out_ps = [psum_o.tile([128, BZS], fp32, tag=f"out{m}",
                                  name=f"out_ps{m}") for m in range(2)]

            def wcol(t):
                if t < 2:
                    return wq0[:, :, t * 128:(t + 1) * 128]
                return wqr[:, :, (t - 2) * 128:(t - 1) * 128]

            def logits(tiles, tag):
                lp = psum_l.tile([128, 2 * BZS], fp32, tag=tag,
                                 name=f"l_{tiles[0]}")
                for k, t in enumerate(tiles):
                    nc.tensor.matmul(lp[:, k * BZS:(k + 1) * BZS],
                                     wcol(t), zq, start=True, stop=True,
                                     perf_mode=DR)
                return lp

            def act_group(lp, tiles):
                sl = slice(tiles[0] * BZS, (tiles[-1] + 1) * BZS)
                n = len(tiles) * BZS
                nc.scalar.activation(e_flat[:, sl], lp[:, 0:n], EXP)
                nc.scalar.activation(spa_flat[:, sl], e_flat[:, sl], LN,
                                     bias=1.0)

            def dve_group(lp, tiles):
                k0 = (tiles[0] - DVE_T0) * BZS
                k1 = (tiles[-1] + 1 - DVE_T0) * BZS
                n = len(tiles) * BZS
                sl = slice(k0, k1)
                # DVE cannot dual-read PSUM: cast to SBUF bf16 first, then
                # square / affine / multiply in fast 2x/4x bf16 modes.
                nc.vector.tensor_copy(lb_flat[:, sl], lp[:, 0:n])
                nc.vector.tensor_tensor(u_flat[:, sl], lb_flat[:, sl],
                                        lb_flat[:, sl], MULT)
                nc.vector.tensor_scalar(r_flat[:, sl], u_flat[:, sl],
                                        C2, C1, MULT, ADD)
                nc.vector.tensor_tensor(spd_flat[:, sl], r_flat[:, sl],
                                        u_flat[:, sl], MULT)

            def main_act_pair(tp, last=False):
                # DoubleRow fp8 contraction over tile pair (tp, tp+1)
                for m in range(2):
                    nc.tensor.matmul(out_ps[m],
                                     uva[:, tp:tp + 2, m * 128:(m + 1) * 128],
                                     spa[:, tp:tp + 2, :],
                                     start=False, stop=last, perf_mode=DR)

            def main_act_single(t, last=False):
                for m in range(2):
                    nc.tensor.matmul(out_ps[m],
                                     uva[:, t, m * 128:(m + 1) * 128],
                                     spa[:, t, :], start=False, stop=last)

            def main_dve(tiles, last=False):
                for t in tiles:
                    for m in range(2):
                        fin = last and t == tiles[-1]
                        nc.tensor.matmul(out_ps[m],
                                         uvd[:, t - DVE_T0,
                                             m * 128:(m + 1) * 128],
                                         spd[:, t - DVE_T0, :],
                                         start=False, stop=fin)

            # ---- schedule ----
            g = {i: GROUPS[i] for i in range(7)}
            lps = {}
            lps[0] = logits(g[0][0], g[0][1])          # A0 tiles (0,1)
            lps[1] = logits(g[1][0], g[1][1])          # D0 tiles (7,8)
            act_group(lps[0], g[0][0])
            lps[2] = logits(g[2][0], g[2][1])          # A1 (2,3)
            dve_group(lps[1], g[1][0])
            lps[3] = logits(g[3][0], g[3][1])          # D1 (9,10)
            # linear term opens the output accumulation group
            for m in range(2):
                nc.tensor.matmul(out_ps[m], gp[:, m * 128:(m + 1) * 128],
                                 zp, start=True, stop=False)
            main_act_pair(0)
            act_group(lps[2], g[2][0])
            dve_group(lps[3], g[3][0])
            lps[4] = logits(g[4][0], g[4][1])          # D2 (11,12)
            main_dve(g[1][0])
            dve_group(lps[4], g[4][0])
            lps[5] = logits(g[5][0], g[5][1])          # A2 (4,5)
            main_act_pair(2)
            act_group(lps[5], g[5][0])
            lps[6] = logits(g[6][0], g[6][1])          # A3 (6,)
            act_group(lps[6], g[6][0])
            main_dve(g[3][0])
            main_dve(g[4][0])
            main_act_pair(4)
            main_act_single(6, last=True)

            # ---- evict (ACT + DVE copies in parallel, two DMA queues) ----
            o0 = singles.tile([128, BZS], fp32)
            nc.scalar.copy(o0, out_ps[0])
            nc.sync.dma_start(out=d_out[0:128, :], in_=o0)
            o1 = singles.tile([128, BZS], fp32)
            nc.vector.tensor_copy(o1, out_ps[1])
            nc.scalar.dma_start(out=d_out[128:256, :], in_=o1)

    nc.compile()
    return nc


def _host_prep(x, z, W, b, tree):
    x = np.asarray(x, dtype=np.float32)
    z = np.asarray(z, dtype=np.float32)
    W = np.asarray(W, dtype=np.float32)
    b = np.asarray(b, dtype=np.float32)
    tree = np.asarray(tree, dtype=np.int64)

    root = tree < 0
    xt = x[:, tree]              # -1 wraps to last column, same as the ref
    xt[:, root] = 1.0            # root fix folded into coefficients

    # DVE-assigned features: rows 2j+s for tiles 7..12 -> j in [448, 784)
    j_dve0 = DVE_T0 * 128 // 2   # 448
    n_dve = F - j_dve0           # real DVE features

    # Ahat over flat rows r=2j+s: xc_s * x, with x -> (x - 1/2) on DVE rows
    # (folds the l/2 part of softplus); -ln2 per DVE feature into h.
    xf = x.copy()
    xf[:, j_dve0:] -= 0.5
    Ahat = np.empty((BX, R), dtype=np.float32)
    Ahat[:, 0::2] = (1.0 - xt) * xf
    Ahat[:, 1::2] = xt * xf
    G = Ahat @ W.T               # [BX, ZD]
    h = Ahat @ b - np.log(2.0) * n_dve

    gp = np.zeros((ZD + 1, BX), dtype=np.float32)
    gp[:ZD] = G.T
    gp[ZD] = h
    gp = gp.astype(BF16)

    # wp: [66, 1664] -- W columns in flat (j, s) order; bias row 64; then
    # fold rows 2p+k into the DoubleRow layout wq[p, k, col].
    wp = np.zeros((2 * KD, RP), dtype=np.float32)
    wp[:ZD, :R] = W
    wp[ZD, :R] = b
    wq = np.ascontiguousarray(wp.reshape(KD, 2, RP)).astype(FP8)

    # uv coefficients: row 2j+s -> U=xt'-1 (s=0) / V=-xt' (s=1)
    UV = np.zeros((RP, BX), dtype=np.float32)
    UV[0:R:2] = xt.T - 1.0
    UV[1:R:2] = -xt.T
    uvm = np.ascontiguousarray(UV.reshape(NT, 128, BX).transpose(1, 0, 2))
    uva = uvm[:, :N_ACT_TILES].astype(FP8)
    uvd = np.ascontiguousarray(uvm[:, N_ACT_TILES:]).astype(BF16)

    # z': [66, 4096] with ones row 64, zero row 65; bf16 flat + fp8 DR forms
    zfull = np.zeros((2 * KD, BZ), dtype=np.float32)
    zfull[:ZD] = z.T
    zfull[ZD] = 1.0
    zp = zfull[:ZD + 1].astype(BF16)
    zq = np.ascontiguousarray(zfull.reshape(KD, 2, BZ)).astype(FP8)

    rep = {"wq0": np.ascontiguousarray(wq[:, :, 0:256]),
           "wqr": np.ascontiguousarray(wq[:, :, 256:]),
           "gp": gp, "uva": uva, "uvd": uvd}
    in_maps = []
    for c in range(N_CORES):
        m = dict(rep)
        m["zp"] = np.ascontiguousarray(zp[:, c * BZS:(c + 1) * BZS])
        m["zq"] = np.ascontiguousarray(zq[:, :, c * BZS:(c + 1) * BZS])
        in_maps.append(m)
    return in_maps


def kernel(x, z, W, b, tree, **_unused):
    import os
    from concourse.bass_utils import run_bass_kernel_spmd

    if "nc" not in _CACHE:
        _CACHE["nc"] = _build_bass()
    nc = _CACHE["nc"]

    in_maps = _host_prep(x, z, W, b, tree)
    res = run_bass_kernel_spmd(nc, in_maps, core_ids=list(range(N_CORES)),
                               tmpdir=os.environ.get("BASS_TMPDIR") or None)
    _CACHE["last_result"] = res
    out = np.concatenate([res.results[c]["out"] for c in range(N_CORES)], axis=1)
    return out.astype(np.float32)


# revision 17
# speedup vs baseline: 1.0462x; 1.0462x over previous
"""Trainium2 Bass kernel for nn_CLTBernoulliDecoder (CLT Bernoulli decoder loss).

Reference computation:
    logits = (z @ W + b).reshape(Bz, F, 2)        # (j, s) column-interleaved
    root fix: logits[:, root, 0] := logits[:, root, 1]
    xt = x[:, tree] ;  x_cond = stack([1-xt, xt])
    ls, lsn = log_sigmoid(+-logits)
    out[b,i] = sum_{j,s} x_cond*x * ls + x_cond*(1-x) * lsn

Algebraic restructuring (exact):
    log_sigmoid(t) = t - softplus(t), log_sigmoid(-t) = -softplus(t)
    =>  out[b,i] = sum_r Ahat[b,r]*l_r[i]  -  sum_r xc[b,r]*softplus(l_r[i])
    over flat rows r = 2j+s (natural W column order), with
    xc[b,2j+s] = x_cond[b,j,s], Ahat = xc*x, root fix folded as xt'=1.
    The linear term folds through W: G = Ahat@W.T.

Activation split across two engines:
    softplus(l) = ln2 + l/2 + g(l),  g(l) = ln(cosh(l/2)) even in l.
  - ACT rows (tiles 0..6): softplus via Exp then Ln(1+e), emitted in fp8.
  - DVE rows (tiles 7..12): g(l) ~= (c2*u + c1)*u with u = l*l
    (cast from PSUM + square / affine / multiply in 2x/4x bf16 modes);
    the (ln2 + l/2) part is folded host-side into Ahat (x -> x-1/2) and
    a -ln2*n constant into h.  Fit |l| <= 2.59: max err 6e-3, typ 1.5e-4.

Tensor engine in fp8 where it is cheap:
  - logits matmuls use fp8e4m3 z/W in DoubleRow layout [33, 2, *]
    (rows 2p+k; row 64 = bias ones row, row 65 = zero pad) -> 2 rows/cycle.
  - ACT-tile main matmuls contract fp8 sp against fp8 coefficients with
    DoubleRow over tile pairs; DVE-tile mains stay bf16 (g is produced by
    the DVE, which has no fast fp8 write path).
  - The linear term gp.T @ zp stays bf16 for accuracy (h is large).

Sharding: data-parallel over Bz (4096 -> 8 x 512); coefficient matrices
replicated; per-core outputs [256, 512] concatenated on axis 1.
"""

import numpy as np
import ml_dtypes

BF16 = ml_dtypes.bfloat16
FP8 = ml_dtypes.float8_e4m3

BX = 256          # data points
BZ = 4096         # latent samples
ZD = 64           # latent dim
F = 784           # features
R = 2 * F         # flat (j, s) rows = 1568
NT = 13           # row tiles of 128 (1664 padded)
RP = NT * 128
KD = (ZD + 2) // 2  # 33 DoubleRow contraction partitions (rows 0..65)
N_CORES = 8
BZS = BZ // N_CORES  # 512 per core

N_ACT_TILES = 7            # tiles 0..6 -> ACT engine (exp+ln)
DVE_T0 = N_ACT_TILES       # tiles 7..12 -> DVE engine (poly even part)
# g(l) = ln(cosh(l/2)) ~= (C2*u + C1)*u, u = l^2, fit for |l| <= 2.59
C1 = 0.12345821
C2 = -0.00355909

_CACHE = {}


def _build_bass():
    import concourse.bass as bass
    import concourse.mybir as mybir
    import concourse.tile as tile
    from concourse import bacc
    from concourse.hw_specs import get_activation_tables

    fp32 = mybir.dt.float32
    bf16 = mybir.dt.bfloat16
    fp8 = mybir.dt.float8e4
    EXP = mybir.ActivationFunctionType.Exp
    LN = mybir.ActivationFunctionType.Ln
    MULT = mybir.AluOpType.mult
    ADD = mybir.AluOpType.add
    DR = mybir.MatmulPerfMode.DoubleRow

    class _Bacc(bacc.Bacc):
        """Pin Exp and Ln to the one table set holding both, so the table
        is loaded once instead of ping-ponging between per-function sets
        (~1.3us per reload). Table ids are global act_info indices, so the
        full table list must be kept in order."""

        def insert_act_table_loads(self):
            EXPF = mybir.ActivationFunctionType.Exp
            LNF = mybir.ActivationFunctionType.Ln
            has_activation = any(
                isinstance(i, mybir.InstActivation)
                for b in self.main_func.blocks
                for i in b.instructions
            )
            if not has_activation:
                return
            tables = []
            for name, funcs in get_activation_tables(self.m.arch).items():
                if name != "natural_log_exp_and_others":
                    funcs = {f for f in funcs if f not in (EXPF, LNF)}
                tables.append((name, funcs))
            import bass_rust as _bass_rust
            _bass_rust.insert_act_table_loads(self, tables)

    nc = _Bacc(None, target_bir_lowering=False)

    d_zp = nc.dram_tensor("zp", [ZD + 1, BZS], bf16, kind="ExternalInput")
    d_w0 = nc.dram_tensor("w0", [ZD + 1, 256], bf16, kind="ExternalInput")
    d_wr = nc.dram_tensor("wr", [ZD + 1, RP - 256], bf16, kind="ExternalInput")
    d_gp = nc.dram_tensor("gp", [ZD + 1, BX], bf16, kind="ExternalInput")
    d_uva = nc.dram_tensor("uva", [128, N_ACT_TILES, BX], fp8,
                           kind="ExternalInput")
    d_uvd = nc.dram_tensor("uvd", [128, NT - N_ACT_TILES, BX], bf16,
                           kind="ExternalInput")
    d_out = nc.dram_tensor("out", [BX, BZS], fp32, kind="ExternalOutput")

    # tile groups: (tiles, psum_tag, engine)
    GROUPS = [
        ((0, 1), "pA", "act"),
        ((7, 8), "pB", "dve"),
        ((2, 3), "pC", "act"),
        ((9, 10), "pA", "dve"),
        ((11, 12), "pB", "dve"),
        ((4, 5), "pC", "act"),
        ((6,), "pA", "act"),
    ]

    with tile.TileContext(nc) as tc:
        with (
            tc.tile_pool(name="singles", bufs=1) as singles,
            tc.tile_pool(name="psum_l", bufs=1, space="PSUM") as psum_l,
            tc.tile_pool(name="psum_o", bufs=1, space="PSUM") as psum_o,
        ):
            # ---- SBUF staging ----
            wu = singles.tile([128, 256], bf16)
            zp = singles.tile([ZD + 1, BZS], bf16)
            w0 = singles.tile([ZD + 1, 256], bf16)
            wr = singles.tile([ZD + 1, RP - 256], bf16)
            gp = singles.tile([ZD + 1, BX], bf16)
            uva = singles.tile([128, N_ACT_TILES, BX], fp8)
            uvd = singles.tile([128, NT - N_ACT_TILES, BX], bf16)
            e_all = singles.tile([128, N_ACT_TILES, BZS], fp32)
            lb_all = singles.tile([128, NT - N_ACT_TILES, BZS], bf16)
            u_all = singles.tile([128, NT - N_ACT_TILES, BZS], bf16)
            r_all = singles.tile([128, NT - N_ACT_TILES, BZS], bf16)
            spa = singles.tile([128, N_ACT_TILES, BZS], fp8)
            spd = singles.tile([128, NT - N_ACT_TILES, BZS], bf16)
            e_flat = e_all.rearrange("p t i -> p (t i)")
            lb_flat = lb_all.rearrange("p t i -> p (t i)")
            u_flat = u_all.rearrange("p t i -> p (t i)")
            r_flat = r_all.rearrange("p t i -> p (t i)")
            spa_flat = spa.rearrange("p t i -> p (t i)")
            spd_flat = spd.rearrange("p t i -> p (t i)")

            # ---- input DMAs: zp/w0 on separate queues gate the first
            # logits; pool queue carries the bulk needed later ----
            nc.vector.memset(wu, 0.0)  # DVE is idle: warmup unblocks first
            nc.sync.dma_start(out=zp, in_=d_zp[:])
            nc.scalar.dma_start(out=w0, in_=d_w0[:])
            nc.gpsimd.dma_start(out=wr, in_=d_wr[:])
            nc.gpsimd.dma_start(out=gp, in_=d_gp[:])
            nc.gpsimd.dma_start(out=uva, in_=d_uva[:])
            nc.gpsimd.dma_start(out=uvd, in_=d_uvd[:])

            # ---- PE warm-up while DMAs land (trips the HAM clock gate) ----
            wu_ps = psum_l.tile([128, 2 * BZS], fp32, tag="pC", name="wu_ps")
            for _ in range(6):
                nc.tensor.matmul(wu_ps[:, 0:256], wu[:, 0:128], wu,
                                 start=True, stop=True)

            out_ps = [psum_o.tile([128, BZS], fp32, tag=f"out{m}",
                                  name=f"out_ps{m}") for m in range(2)]

            def wcol(t):
                if t < 2:
                    return w0[:, t * 128:(t + 1) * 128]
                return wr[:, (t - 2) * 128:(t - 1) * 128]

            def logits(tiles, tag):
                lp = psum_l.tile([128, 2 * BZS], fp32, tag=tag,
                                 name=f"l_{tiles[0]}")
                for k, t in enumerate(tiles):
                    nc.tensor.matmul(lp[:, k * BZS:(k + 1) * BZS],
                                     wcol(t), zp, start=True, stop=True)
                return lp

            def act_group(lp, tiles):
                sl = slice(tiles[0] * BZS, (tiles[-1] + 1) * BZS)
                n = len(tiles) * BZS
                nc.scalar.activation(e_flat[:, sl], lp[:, 0:n], EXP)
                nc.scalar.activation(spa_flat[:, sl], e_flat[:, sl], LN,
                                     bias=1.0)

            def dve_group(lp, tiles):
                k0 = (tiles[0] - DVE_T0) * BZS
                k1 = (tiles[-1] + 1 - DVE_T0) * BZS
                n = len(tiles) * BZS
                sl = slice(k0, k1)
                # DVE cannot dual-read PSUM: cast to SBUF bf16 first, then
                # square / affine / multiply in fast 2x/4x bf16 modes.
                nc.vector.tensor_copy(lb_flat[:, sl], lp[:, 0:n])
                nc.vector.tensor_tensor(u_flat[:, sl], lb_flat[:, sl],
                                        lb_flat[:, sl], MULT)
                nc.vector.tensor_scalar(r_flat[:, sl], u_flat[:, sl],
                                        C2, C1, MULT, ADD)
                nc.vector.tensor_tensor(spd_flat[:, sl], r_flat[:, sl],
                                        u_flat[:, sl], MULT)

            def main_act_pair(tp, last=False):
                # DoubleRow fp8 contraction over tile pair (tp, tp+1)
                for m in range(2):
                    nc.tensor.matmul(out_ps[m],
                                     uva[:, tp:tp + 2, m * 128:(m + 1) * 128],
                                     spa[:, tp:tp + 2, :],
                                     start=False, stop=last, perf_mode=DR)

            def main_act_single(t, last=False):
                for m in range(2):
                    nc.tensor.matmul(out_ps[m],
                                     uva[:, t, m * 128:(m + 1) * 128],
                                     spa[:, t, :], start=False, stop=last)

            def main_dve(tiles, last=False):
                for t in tiles:
                    for m in range(2):
                        fin = last and t == tiles[-1]
                        nc.tensor.matmul(out_ps[m],
                                         uvd[:, t - DVE_T0,
                                             m * 128:(m + 1) * 128],
                                         spd[:, t - DVE_T0, :],
                                         start=False, stop=fin)

            # ---- schedule ----
            g = {i: GROUPS[i] for i in range(7)}
            lps = {}
            lps[0] = logits(g[0][0], g[0][1])          # A0 tiles (0,1)
            lps[1] = logits(g[1][0], g[1][1])          # D0 tiles (7,8)
            act_group(lps[0], g[0][0])
            lps[2] = logits(g[2][0], g[2][1])          # A1 (2,3)
            dve_group(lps[1], g[1][0])
            lps[3] = logits(g[3][0], g[3][1])          # D1 (9,10)
            # linear term opens the output accumulation group
            for m in range(2):
                nc.tensor.matmul(out_ps[m], gp[:, m * 128:(m + 1) * 128],
                                 zp, start=True, stop=False)
            main_act_pair(0)
            act_group(lps[2], g[2][0])
            dve_group(lps[3], g[3][0])
            lps[4] = logits(g[4][0], g[4][1])          # D2 (11,12)
            main_dve(g[1][0])
            dve_group(lps[4], g[4][0])
            lps[5] = logits(g[5][0], g[5][1])          # A2 (4,5)
            main_act_pair(2)
            act_group(lps[5], g[5][0])
            lps[6] = logits(g[6][0], g[6][1])          # A3 (6,)
            act_group(lps[6], g[6][0])
            main_dve(g[3][0])
            main_dve(g[4][0])
            main_act_pair(4)
            main_act_single(6, last=True)

            # ---- evict (ACT + DVE copies in parallel, two DMA queues) ----
            o0 = singles.tile([128, BZS], fp32)
            nc.scalar.copy(o0, out_ps[0])
            nc.sync.dma_start(out=d_out[0:128, :], in_=o0)
            o1 = singles.tile([128, BZS], fp32)
            nc.vector.tensor_copy(o1, out_ps[1])
            nc.scalar.dma_start(out=d_out[128:256, :], in_=o1)

    nc.compile()
    return nc


def _host_prep(x, z, W, b, tree):
    x = np.asarray(x, dtype=np.float32)
    z = np.asarray(z, dtype=np.float32)
    W = np.asarray(W, dtype=np.float32)
    b = np.asarray(b, dtype=np.float32)
    tree = np.asarray(tree, dtype=np.int64)

    root = tree < 0
    xt = x[:, tree]              # -1 wraps to last column, same as the ref
    xt[:, root] = 1.0            # root fix folded into coefficients

    # DVE-assigned features: rows 2j+s for tiles 7..12 -> j in [448, 784)
    j_dve0 = DVE_T0 * 128 // 2   # 448
    n_dve = F - j_dve0           # real DVE features

    # Ahat over flat rows r=2j+s: xc_s * x, with x -> (x - 1/2) on DVE rows
    # (folds the l/2 part of softplus); -ln2 per DVE feature into h.
    xf = x.copy()
    xf[:, j_dve0:] -= 0.5
    Ahat = np.empty((BX, R), dtype=np.float32)
    Ahat[:, 0::2] = (1.0 - xt) * xf
    Ahat[:, 1::2] = xt * xf
    G = Ahat @ W.T               # [BX, ZD]
    h = Ahat @ b - np.log(2.0) * n_dve

    gp = np.zeros((ZD + 1, BX), dtype=np.float32)
    gp[:ZD] = G.T
    gp[ZD] = h
    gp = gp.astype(BF16)

    # wp: [65, 1664] -- W columns already in flat (j, s) order; bias row 64
    wp = np.zeros((ZD + 1, RP), dtype=np.float32)
    wp[:ZD, :R] = W
    wp[ZD, :R] = b
    wp = wp.astype(BF16)

    # uv coefficients: row 2j+s -> U=xt'-1 (s=0) / V=-xt' (s=1)
    UV = np.zeros((RP, BX), dtype=np.float32)
    UV[0:R:2] = xt.T - 1.0
    UV[1:R:2] = -xt.T
    uvm = np.ascontiguousarray(UV.reshape(NT, 128, BX).transpose(1, 0, 2))
    uva = uvm[:, :N_ACT_TILES].astype(FP8)
    uvd = np.ascontiguousarray(uvm[:, N_ACT_TILES:]).astype(BF16)

    # z': [65, 4096] with ones row (bias channel)
    zp = np.ones((ZD + 1, BZ), dtype=np.float32)
    zp[:ZD] = z.T
    zp = zp.astype(BF16)

    rep = {"w0": np.ascontiguousarray(wp[:, 0:256]),
           "wr": np.ascontiguousarray(wp[:, 256:]),
           "gp": gp, "uva": uva, "uvd": uvd}
    in_maps = []
    for c in range(N_CORES):
        m = dict(rep)
        m["zp"] = np.ascontiguousarray(zp[:, c * BZS:(c + 1) * BZS])
        in_maps.append(m)
    return in_maps


def kernel(x, z, W, b, tree, **_unused):
    import os
    from concourse.bass_utils import run_bass_kernel_spmd

    if "nc" not in _CACHE:
        _CACHE["nc"] = _build_bass()
    nc = _CACHE["nc"]

    in_maps = _host_prep(x, z, W, b, tree)
    res = run_bass_kernel_spmd(nc, in_maps, core_ids=list(range(N_CORES)),
                               tmpdir=os.environ.get("BASS_TMPDIR") or None)
    _CACHE["last_result"] = res
    out = np.concatenate([res.results[c]["out"] for c in range(N_CORES)], axis=1)
    return out.astype(np.float32)
